# revision 23
# baseline (speedup 1.0000x reference)
"""Trainium2 Bass kernel for a Bahdanau-attention decoder step.

Computes, for B=16, L=4096, A=512, H=512:
    dec  = concat(h, c) @ W.T + b                      # [B, A]
    e    = sum_a v[a] * tanh(feat[b,l,a] + dec[b,a])   # [B, L]
    attn = softmax(e) * mask, renormalized             # [B, L]
    ctx  = sum_l attn[b,l] * state[b,l,a]              # [B, A]

Sharding: data-parallel over batch B across 8 NeuronCores (2 rows/core).
The tiny dec projection (16x1024 @ 1024x512) is done host-side; dec, v and
mask are passed in device-friendly layouts so the kernel streams the two
134MB tensors exactly once each (memory-bound target).

Device dataflow per core (2 batch rows, 4 L-groups of 1024 per row, each
group = 8 subtiles of 128 L x 512 A, natural layout: L on partitions):
  DMA   : 2MB contiguous loads of feat/state groups
  PE    : X = dec (K=1 outer product) + feat (identity matmul) into PSUM
          -- the broadcast add costs no DVE/ACT work
  ACT   : T = tanh(X) PSUM->SBUF (also the PSUM evacuation)
  DVE   : fused tensor_tensor_reduce: e[:,u] = sum_a T * v_bcast
          -- e lands directly in column layout [128L, 1]
  ACT   : w = exp(e)  (no max-subtraction: |e| <~ 40 so fp32 exp is safe)
  DVE   : W_sb[:, cols] = w * maskT
  PE    : ctx += W_sb[:,u].T @ state subtile
  finale: denom via transpose+reduce, reciprocal, scale, transpose attn out.

This walrus build accepts only ONE semaphore wait per instruction, so the
trace order and tile reuse are arranged to keep every instruction at one
cross-engine wait (see _split_multiwaits for the general fallback).
"""

import os
import sys

import numpy as np

sys.path.insert(0, "/opt/trn_rl_repo")

B, L, A, H = 16, 4096, 512, 512
NCORES = 8
BPC = B // NCORES      # batch rows per core
G = 4                  # L-groups per batch row (1024 L each, 2MB per DMA)
J = 8                  # 128-L subtiles per group
U = G * J              # 32 subtiles per batch row
P = 128
NX = 4                 # rotating PSUM tiles for the add/tanh stage

_CACHE = {}
LAST_RESULTS = None    # BassKernelResults of the most recent run (for test.py)
TRACE = False


def _build_nc():
    import concourse.bass as bass
    import concourse.tile as tile
    from concourse import masks, mybir
    from concourse.tile import add_dep_helper

    f32 = mybir.dt.float32
    Act = mybir.ActivationFunctionType

    nc = bass.Bass()

    feat = nc.dram_tensor("feat", [BPC, L, A], f32, kind="ExternalInput")
    state = nc.dram_tensor("state", [BPC, L, A], f32, kind="ExternalInput")
    dec = nc.dram_tensor("dec", [1, BPC * A], f32, kind="ExternalInput")
    vrow = nc.dram_tensor("vrow", [1, A], f32, kind="ExternalInput")
    maskT = nc.dram_tensor("maskT", [P, BPC * U], f32, kind="ExternalInput")
    ctx_out = nc.dram_tensor("ctx", [BPC, A], f32, kind="ExternalOutput")
    attn_out = nc.dram_tensor("attn", [BPC, L], f32, kind="ExternalOutput")

    # group g holds L rows 1024g..1024g+1023; partition p carries row
    # 1024g + 128j + p
    featR = feat.rearrange("b (g j p) a -> b g p j a", g=G, j=J, p=P)
    stateR = state.rearrange("b (g j p) a -> b g p j a", g=G, j=J, p=P)
    attnR = attn_out.rearrange("b (u q) -> b u q", u=U)

    with tile.TileContext(nc) as tc:
        with (
            tc.tile_pool(name="consts", bufs=1) as consts,
            tc.tile_pool(name="fpool", bufs=3) as fpool,
            tc.tile_pool(name="spool", bufs=3) as spool,
            tc.tile_pool(name="tanhp", bufs=6) as tanhp,
            tc.tile_pool(name="junkp", bufs=2) as junkp,
            tc.tile_pool(name="wp", bufs=4) as wp,
            tc.tile_pool(name="wsbp", bufs=2) as wsbp,
            tc.tile_pool(name="finsb", bufs=2) as finsb,
            tc.tile_pool(name="xps", bufs=1, space="PSUM") as xps,
            tc.tile_pool(name="ctxps", bufs=1, space="PSUM") as ctxps,
            tc.tile_pool(name="finps", bufs=1, space="PSUM") as finps,
        ):
            ident = consts.tile([P, P], f32)
            masks.make_identity(nc, ident[:])
            ones_col = consts.tile([P, 1], f32)
            nc.gpsimd.memset(ones_col[:], 1.0)
            ones_row = consts.tile([1, P], f32)
            nc.gpsimd.memset(ones_row[:], 1.0)

            dec_sb = consts.tile([1, BPC * A], f32)
            nc.sync.dma_start(out=dec_sb[:], in_=dec[:])
            v_sb = consts.tile([1, A], f32)
            nc.sync.dma_start(out=v_sb[:], in_=vrow[:])
            maskT_sb = consts.tile([P, BPC * U], f32)
            nc.sync.dma_start(out=maskT_sb[:], in_=maskT[:])

            # PSUM tiles, allocated once and reused (same-tile hazards stay
            # in PE program order; pool slot recycling would add release
            # waits that overflow the 1-wait instruction budget).
            xtiles = [
                xps.tile([P, A], f32, tag=f"x{i}", name=f"x{i}") for i in range(NX)
            ]
            ctx_ps = ctxps.tile([1, A], f32, tag="ctx")
            fin_ps = finps.tile([P, 1 + 2 * P], f32, tag="fin")

            # Engine observers (see module docstring): each engine touches
            # the DMA'd constants once so later instructions carry only
            # their data wait.
            sc_d = consts.tile([P, 1], f32)
            nc.vector.tensor_copy(sc_d[:], maskT_sb[:, :1])   # DVE <- maskT
            # PE observers: gpsimd consts, then the dec DMA
            nc.tensor.matmul(
                fin_ps[:1, 0:1], ones_col[:], ones_col[:], start=True, stop=True
            )
            nc.tensor.transpose(fin_ps[:, 1:2], dec_sb[:1, :P], ident[:1, :1])

            # v broadcast to all partitions: K=1 outer products into the
            # fin_ps scratch, evacuated by ACT (which thereby observes PE).
            Q = P  # 128-column chunks of v
            v_bc = consts.tile([P, A], f32)
            for q in range(A // Q):
                nc.tensor.matmul(
                    fin_ps[:, 1 : 1 + Q],
                    ones_row[:],
                    v_sb[:1, Q * q : Q * (q + 1)],
                    start=True,
                    stop=True,
                )
                nc.scalar.copy(v_bc[:, Q * q : Q * (q + 1)], fin_ps[:, 1 : 1 + Q])

            # last tanh reader of each xtile, for explicit WAR deps (the
            # scheduler treats PSUM accumulation groups as commutative and
            # otherwise lets a later group's start=True overwrite run first)
            xlast = [None] * NX

            for b in range(BPC):
                W_sb = wsbp.tile([P, U], f32, tag="wsb")
                ecols = wsbp.tile([P, U], f32, tag="ecols")

                if b > 0:
                    # PE observes the previous batch finalization's readers
                    # of ctx_ps / fin_ps before overwriting them.
                    nc.tensor.transpose(ctx_ps[:1, :P], ones_col[:], ident[:])
                    nc.tensor.transpose(
                        fin_ps[:1, 1 + P : 1 + 2 * P], ones_col[:], ident[:]
                    )

                for g in range(G):
                    F_g = fpool.tile([P, J, A], f32, tag="fg")
                    nc.sync.dma_start(out=F_g[:], in_=featR[b, g])
                    S_g = spool.tile([P, J, A], f32, tag="sg")
                    nc.sync.dma_start(out=S_g[:], in_=stateR[b, g])

                    # Half-group at a time: build X = dec + feat in PSUM
                    # for 4 subtiles (stationary operands load once per
                    # pair), then tanh/multiply/reduce them. Trace order
                    # matters: the WAR deps below need the previous tanh of
                    # each PSUM tile to exist already.
                    for half in range(2):
                        for j2 in range(2):
                            ja = 4 * half + 2 * j2
                            jb = ja + 1
                            xa = xtiles[ja % NX]
                            xb = xtiles[jb % NX]
                            da = nc.tensor.matmul(
                                xa[:], ones_row[:], dec_sb[:1, b * A : (b + 1) * A],
                                start=True, stop=False,
                            )
                            db = nc.tensor.matmul(
                                xb[:], ones_row[:], dec_sb[:1, b * A : (b + 1) * A],
                                start=True, stop=False,
                            )
                            # WAR: the overwrite must wait for the previous
                            # tanh read of this PSUM tile (the scheduler
                            # treats accumulation groups as commutative and
                            # would otherwise reorder).
                            if xlast[ja % NX] is not None:
                                add_dep_helper(da.ins, xlast[ja % NX].ins,
                                               reason="xtile WAR vs tanh")
                            if xlast[jb % NX] is not None:
                                add_dep_helper(db.ins, xlast[jb % NX].ins,
                                               reason="xtile WAR vs tanh")
                            fa = nc.tensor.matmul(
                                xa[:], ident[:], F_g[:, ja, :],
                                start=False, stop=True,
                            )
                            fb = nc.tensor.matmul(
                                xb[:], ident[:], F_g[:, jb, :],
                                start=False, stop=True,
                            )
                            # start=True must execute before the accumulate
                            add_dep_helper(fa.ins, da.ins, sync=False,
                                           reason="psum accum order")
                            add_dep_helper(fb.ins, db.ins, sync=False,
                                           reason="psum accum order")

                        for j in range(4 * half, 4 * half + 4):
                            u = J * g + j
                            T_j = tanhp.tile(
                                [P, A], f32, tag="th", name=f"th{b}_{g}_{j}"
                            )
                            xlast[j % NX] = nc.scalar.activation(
                                T_j[:], xtiles[j % NX][:], Act.Tanh
                            )
                            junk = junkp.tile([P, A], f32, tag="junk")
                            nc.vector.tensor_mul(junk[:], T_j[:], v_bc[:])
                            if j % 2 == 0:
                                nc.vector.tensor_reduce(
                                    ecols[:, u : u + 1],
                                    junk[:],
                                    mybir.AxisListType.X,
                                    mybir.AluOpType.add,
                                )
                            else:
                                nc.scalar.activation(
                                    junk[:],
                                    junk[:],
                                    Act.Identity,
                                    accum_out=ecols[:, u : u + 1],
                                )

                    # exp + mask for this group's 8 columns
                    wexp = wp.tile([P, J], f32, tag="wexp")
                    nc.scalar.activation(
                        wexp[:], ecols[:, J * g : J * (g + 1)], Act.Exp
                    )
                    nc.vector.tensor_mul(
                        W_sb[:, J * g : J * (g + 1)],
                        wexp[:],
                        maskT_sb[:, U * b + J * g : U * b + J * (g + 1)],
                    )

                    # PE touch of the DVE write so the ctx matmuls carry only
                    # the state-DMA wait.
                    nc.tensor.transpose(
                        fin_ps[:J, 1 : 1 + P],
                        W_sb[:, J * g : J * (g + 1)],
                        ident[:],
                    )

                    for j in range(J):
                        u = J * g + j
                        nc.tensor.matmul(
                            ctx_ps[:],
                            W_sb[:, u : u + 1],
                            S_g[:, j, :],
                            start=(u == 0),
                            stop=(u == U - 1),
                        )

                # ---- batch-row finalization ----
                # fin_ps regions: col 0 = reciprocal broadcast, cols 1..128 =
                # rowsum transpose (also scratch), cols 129..256 = attn
                # transpose.
                rowsum = finsb.tile([P, 1], f32, tag="rowsum")
                nc.vector.tensor_reduce(
                    rowsum[:], W_sb[:], mybir.AxisListType.X, mybir.AluOpType.add
                )
                nc.tensor.transpose(fin_ps[:1, 1 : 1 + P], rowsum[:], ident[:])
                denom = finsb.tile([1, 1], f32, tag="denom")
                nc.vector.tensor_reduce(
                    denom[:], fin_ps[:1, 1 : 1 + P], mybir.AxisListType.X,
                    mybir.AluOpType.add,
                )
                recip = finsb.tile([1, 1], f32, tag="recip")
                nc.vector.reciprocal(recip[:], denom[:])

                # broadcast 1/denom to all partitions via K=1 outer product
                nc.tensor.matmul(
                    fin_ps[:, 0:1], ones_row[:], recip[:], start=True, stop=True
                )
                rb = finsb.tile([P, 1], f32, tag="rb")
                nc.vector.tensor_copy(rb[:], fin_ps[:, 0:1])

                attn_scaled = finsb.tile([P, U], f32, tag="ascaled")
                nc.vector.tensor_scalar_mul(attn_scaled[:], W_sb[:], rb[:])

                nc.tensor.transpose(
                    fin_ps[:U, 1 + P : 1 + 2 * P], attn_scaled[:], ident[:]
                )
                nc.scalar.copy(sc_d[:1, :1], recip[:])  # ACT observes DVE recip
                attn_sb = finsb.tile([U, P], f32, tag="asb")
                nc.scalar.copy(attn_sb[:], fin_ps[:U, 1 + P : 1 + 2 * P])
                nc.sync.dma_start(out=attnR[b], in_=attn_sb[:])

                ctx_sb = finsb.tile([1, A], f32, tag="ctxsb")
                nc.scalar.mul(ctx_sb[:], ctx_ps[:], recip[:1, :1])
                nc.sync.dma_start(out=ctx_out[b : b + 1, :], in_=ctx_sb[:])

    _split_multiwaits(nc)
    return nc


def _split_multiwaits(nc):
    """Walrus in this toolchain accepts only ONE semaphore wait per
    instruction. Tile occasionally emits more (data dep + hazard dep on
    another engine). Splitting is semantics-preserving: engine streams
    execute in order, so hoisting extra waits onto same-engine NoOps
    immediately before the instruction blocks identically."""
    import concourse.mybir as mybir

    n_split = 0
    for fn in nc.m.functions:
        for blk in fn.blocks:
            insts = blk.instructions
            i = 0
            while i < len(insts):
                inst = insts[i]
                si = getattr(inst, "sync_info", None)
                eng = getattr(inst, "engine", None)
                engname = str(eng).split(".")[-1] if eng is not None else ""
                if (
                    si is not None
                    and si.on_wait
                    and len(si.on_wait) > 1
                    and engname in ("Activation", "PE", "DVE", "Pool", "SP")
                    and type(inst).__name__ != "InstISA"
                ):
                    waits = list(si.on_wait)
                    for k, w in enumerate(waits[:-1]):
                        nop = mybir.InstNoOp(name=f"{inst.name}-ws{k}", engine=eng)
                        nop.sync_info = mybir.SyncInfo(on_wait=[w], on_update=[])
                        insts.insert(i, nop)
                        i += 1
                    inst.sync_info = mybir.SyncInfo(
                        on_wait=[waits[-1]], on_update=list(si.on_update or [])
                    )
                    n_split += 1
                i += 1
    return nc


def _get_nc():
    if "nc" not in _CACHE:
        _CACHE["nc"] = _build_nc()
    return _CACHE["nc"]


def kernel(encoder_features, h, c, encoder_state, encoder_mask, v, W, b):
    global LAST_RESULTS
    from concourse.bass_utils import run_bass_kernel_spmd

    ef = np.ascontiguousarray(np.asarray(encoder_features, np.float32)).reshape(B, L, A)
    es = np.ascontiguousarray(np.asarray(encoder_state, np.float32)).reshape(B, L, A)
    h = np.asarray(h, np.float32)
    c = np.asarray(c, np.float32)
    mask = np.asarray(encoder_mask, np.float32)
    v = np.asarray(v, np.float32)
    W = np.asarray(W, np.float32)
    bb = np.asarray(b, np.float32)

    # dec = [h, c] @ W.T + b  (tiny: 16x1024 @ 1024x512)
    dec = np.concatenate([h, c], axis=1) @ W.T + bb  # [B, A]
    vr = np.ascontiguousarray(v.reshape(1, A))

    in_maps = []
    for k in range(NCORES):
        sl = slice(BPC * k, BPC * (k + 1))
        # maskT[p, U*b + u] = mask[b, 128u + p]
        maskT = np.ascontiguousarray(
            np.concatenate(
                [mask[BPC * k + i].reshape(U, P).T for i in range(BPC)], axis=1
            )
        )
        in_maps.append(
            {
                "feat": np.ascontiguousarray(ef[sl]),
                "state": np.ascontiguousarray(es[sl]),
                "dec": np.ascontiguousarray(dec[sl].reshape(1, BPC * A)),
                "vrow": vr,
                "maskT": maskT,
            }
        )

    nc = _get_nc()
    res = run_bass_kernel_spmd(
        nc,
        in_maps,
        core_ids=list(range(NCORES)),
        trace=TRACE or bool(int(os.environ.get("KERNEL_TRACE", "0"))),
    )
    LAST_RESULTS = res

    context = np.concatenate([r["ctx"] for r in res.results], axis=0)
    attn = np.concatenate([r["attn"] for r in res.results], axis=0)
    return context, attn


# revision 24
# speedup vs baseline: 1.4660x; 1.4660x over previous
"""Trainium2 Bass kernel for a Bahdanau-attention decoder step.

Computes, for B=16, L=4096, A=512, H=512:
    dec  = concat(h, c) @ W.T + b                      # [B, A]
    e    = sum_a v[a] * tanh(feat[b,l,a] + dec[b,a])   # [B, L]
    attn = softmax(e) * mask, renormalized             # [B, L]
    ctx  = sum_l attn[b,l] * state[b,l,a]              # [B, A]

Sharding: data-parallel over batch B across 8 NeuronCores (2 rows/core).
The tiny dec projection (16x1024 @ 1024x512) is done host-side; dec, v and
mask are passed in device-friendly layouts so the kernel streams the two
134MB tensors exactly once each (memory-bound target).

Device dataflow per core (2 batch rows, 4 L-groups of 1024 per row, each
group = 8 subtiles of 128 L x 512 A, natural layout: L on partitions):
  DMA   : 2MB contiguous loads of feat/state groups
  DVE   : X = feat + dec_bcast (dec broadcast materialized once per batch
          via a K=1 PE outer product)
  ACT   : T = tanh(X)
  DVE   : prod = T * v_bcast (in-place over X)
  ACT   : e[:,u] = accum_out of an Identity activation over prod
          -- e lands directly in column layout [128L, 1]
  ACT   : w = exp(e)  (no max-subtraction: |e| <~ 40 so fp32 exp is safe)
  DVE   : W_sb[:, cols] = w * maskT
  PE    : ctx += W_sb[:,u].T @ state subtile
  finale: denom via transpose+reduce, reciprocal, scale, transpose attn out.

This walrus build accepts only ONE semaphore wait per instruction, so the
trace order and tile reuse are arranged to keep every instruction at one
cross-engine wait (see _split_multiwaits for the general fallback).
"""

import os
import sys

import numpy as np

sys.path.insert(0, "/opt/trn_rl_repo")

B, L, A, H = 16, 4096, 512, 512
NCORES = 8
BPC = B // NCORES      # batch rows per core
G = 4                  # L-groups per batch row (1024 L each, 2MB per DMA)
J = 8                  # 128-L subtiles per group
U = G * J              # 32 subtiles per batch row
P = 128

_CACHE = {}
LAST_RESULTS = None    # BassKernelResults of the most recent run (for test.py)
TRACE = False


def _build_nc():
    import concourse.bass as bass
    import concourse.tile as tile
    from concourse import masks, mybir
    from concourse.tile import add_dep_helper

    f32 = mybir.dt.float32
    Act = mybir.ActivationFunctionType

    nc = bass.Bass()

    feat = nc.dram_tensor("feat", [BPC, L, A], f32, kind="ExternalInput")
    state = nc.dram_tensor("state", [BPC, L, A], f32, kind="ExternalInput")
    dec = nc.dram_tensor("dec", [1, BPC * A], f32, kind="ExternalInput")
    vrow = nc.dram_tensor("vrow", [1, A], f32, kind="ExternalInput")
    maskT = nc.dram_tensor("maskT", [P, BPC * U], f32, kind="ExternalInput")
    ctx_out = nc.dram_tensor("ctx", [BPC, A], f32, kind="ExternalOutput")
    attn_out = nc.dram_tensor("attn", [BPC, L], f32, kind="ExternalOutput")

    # group g holds L rows 1024g..1024g+1023; partition p carries row
    # 1024g + 128j + p
    featR = feat.rearrange("b (g j p) a -> b g p j a", g=G, j=J, p=P)
    stateR = state.rearrange("b (g j p) a -> b g p j a", g=G, j=J, p=P)
    attnR = attn_out.rearrange("b (u q) -> b u q", u=U)

    with tile.TileContext(nc) as tc:
        with (
            tc.tile_pool(name="consts", bufs=1) as consts,
            tc.tile_pool(name="fpool", bufs=3) as fpool,
            tc.tile_pool(name="spool", bufs=3) as spool,
            tc.tile_pool(name="xsp", bufs=4) as xsp,
            tc.tile_pool(name="tanhp", bufs=4) as tanhp,
            tc.tile_pool(name="wp", bufs=4) as wp,
            tc.tile_pool(name="wsbp", bufs=2) as wsbp,
            tc.tile_pool(name="finsb", bufs=2) as finsb,
            tc.tile_pool(name="bcps", bufs=1, space="PSUM") as bcps,
            tc.tile_pool(name="ctxps", bufs=1, space="PSUM") as ctxps,
            tc.tile_pool(name="finps", bufs=1, space="PSUM") as finps,
        ):
            ident = consts.tile([P, P], f32)
            masks.make_identity(nc, ident[:])
            ones_col = consts.tile([P, 1], f32)
            nc.gpsimd.memset(ones_col[:], 1.0)
            ones_row = consts.tile([1, P], f32)
            nc.gpsimd.memset(ones_row[:], 1.0)

            dec_sb = consts.tile([1, BPC * A], f32)
            nc.sync.dma_start(out=dec_sb[:], in_=dec[:])
            v_sb = consts.tile([1, A], f32)
            nc.sync.dma_start(out=v_sb[:], in_=vrow[:])
            maskT_sb = consts.tile([P, BPC * U], f32)
            nc.sync.dma_start(out=maskT_sb[:], in_=maskT[:])

            # PSUM tiles, allocated once and reused across batch rows
            bc_ps = bcps.tile([P, A], f32, tag="bc")
            ctx_ps = ctxps.tile([1, A], f32, tag="ctx")
            fin_ps = finps.tile([P, 1 + 2 * P], f32, tag="fin")

            sc_d = consts.tile([P, 1], f32)
            nc.vector.tensor_copy(sc_d[:], maskT_sb[:, :1])   # DVE <- maskT

            # v broadcast to all partitions: K=1 outer product + ACT copy
            v_bc = consts.tile([P, A], f32)
            nc.tensor.matmul(bc_ps[:], ones_row[:], v_sb[:1, :], start=True, stop=True)
            nc.scalar.copy(v_bc[:], bc_ps[:])

            # Cross-batch ordering anchors: the scheduler treats PSUM
            # accumulation groups as commutative, so the next batch's
            # overwrites need explicit deps on the previous readers.
            prev_ctx_read = None
            prev_bc_read = None

            for b in range(BPC):
                W_sb = wsbp.tile([P, U], f32, tag="wsb")
                ecols = wsbp.tile([P, U], f32, tag="ecols")

                # dec broadcast for this batch row
                mm_bc = nc.tensor.matmul(
                    bc_ps[:], ones_row[:], dec_sb[:1, b * A : (b + 1) * A],
                    start=True, stop=True,
                )
                if prev_bc_read is not None:
                    add_dep_helper(mm_bc.ins, prev_bc_read.ins,
                                   reason="bc_ps WAR vs previous copy")
                dec_bc = consts.tile([P, A], f32, tag=f"dec_bc{b}", name=f"dec_bc{b}")
                prev_bc_read = nc.scalar.copy(dec_bc[:], bc_ps[:])

                for g in range(G):
                    F_g = fpool.tile([P, J, A], f32, tag="fg")
                    nc.sync.dma_start(out=F_g[:], in_=featR[b, g])
                    S_g = spool.tile([P, J, A], f32, tag="sg")
                    nc.sync.dma_start(out=S_g[:], in_=stateR[b, g])

                    for j in range(J):
                        u = J * g + j
                        Xs = xsp.tile([P, A], f32, tag="xs")
                        nc.vector.tensor_add(Xs[:], F_g[:, j, :], dec_bc[:])
                        T_j = tanhp.tile([P, A], f32, tag="th")
                        nc.scalar.activation(T_j[:], Xs[:], Act.Tanh)
                        nc.vector.tensor_mul(Xs[:], T_j[:], v_bc[:])
                        nc.scalar.activation(
                            Xs[:], Xs[:], Act.Identity,
                            accum_out=ecols[:, u : u + 1],
                        )

                    # exp + mask for this group's 8 columns
                    wexp = wp.tile([P, J], f32, tag="wexp")
                    nc.scalar.activation(
                        wexp[:], ecols[:, J * g : J * (g + 1)], Act.Exp
                    )
                    nc.vector.tensor_mul(
                        W_sb[:, J * g : J * (g + 1)],
                        wexp[:],
                        maskT_sb[:, U * b + J * g : U * b + J * (g + 1)],
                    )

                    for j in range(J):
                        u = J * g + j
                        mm = nc.tensor.matmul(
                            ctx_ps[:],
                            W_sb[:, u : u + 1],
                            S_g[:, j, :],
                            start=(u == 0),
                            stop=(u == U - 1),
                        )
                        if u == 0 and prev_ctx_read is not None:
                            add_dep_helper(mm.ins, prev_ctx_read.ins,
                                           reason="ctx WAR vs previous read")

                # ---- batch-row finalization ----
                # fin_ps regions: col 0 = reciprocal broadcast, cols 1..128 =
                # rowsum transpose (also scratch), cols 129..256 = attn
                # transpose.
                rowsum = finsb.tile([P, 1], f32, tag="rowsum")
                nc.vector.tensor_reduce(
                    rowsum[:], W_sb[:], mybir.AxisListType.X, mybir.AluOpType.add
                )
                nc.tensor.transpose(fin_ps[:1, 1 : 1 + P], rowsum[:], ident[:])
                denom = finsb.tile([1, 1], f32, tag="denom")
                nc.vector.tensor_reduce(
                    denom[:], fin_ps[:1, 1 : 1 + P], mybir.AxisListType.X,
                    mybir.AluOpType.add,
                )
                recip = finsb.tile([1, 1], f32, tag="recip")
                nc.vector.reciprocal(recip[:], denom[:])

                # broadcast 1/denom to all partitions via K=1 outer product
                nc.tensor.matmul(
                    fin_ps[:, 0:1], ones_row[:], recip[:], start=True, stop=True
                )
                rb = finsb.tile([P, 1], f32, tag="rb")
                nc.vector.tensor_copy(rb[:], fin_ps[:, 0:1])

                attn_scaled = finsb.tile([P, U], f32, tag="ascaled")
                nc.vector.tensor_scalar_mul(attn_scaled[:], W_sb[:], rb[:])

                nc.tensor.transpose(
                    fin_ps[:U, 1 + P : 1 + 2 * P], attn_scaled[:], ident[:]
                )
                nc.scalar.copy(sc_d[:1, :1], recip[:])  # ACT observes DVE recip
                attn_sb = finsb.tile([U, P], f32, tag="asb")
                nc.scalar.copy(attn_sb[:], fin_ps[:U, 1 + P : 1 + 2 * P])
                nc.sync.dma_start(out=attnR[b], in_=attn_sb[:])

                ctx_sb = finsb.tile([1, A], f32, tag="ctxsb")
                prev_ctx_read = nc.scalar.mul(ctx_sb[:], ctx_ps[:], recip[:1, :1])
                nc.sync.dma_start(out=ctx_out[b : b + 1, :], in_=ctx_sb[:])

    _split_multiwaits(nc)
    return nc


def _split_multiwaits(nc):
    """Walrus in this toolchain accepts only ONE semaphore wait per
    instruction. Tile occasionally emits more (data dep + hazard dep on
    another engine). Splitting is semantics-preserving: engine streams
    execute in order, so hoisting extra waits onto same-engine NoOps
    immediately before the instruction blocks identically."""
    import concourse.mybir as mybir

    n_split = 0
    for fn in nc.m.functions:
        for blk in fn.blocks:
            insts = blk.instructions
            i = 0
            while i < len(insts):
                inst = insts[i]
                si = getattr(inst, "sync_info", None)
                eng = getattr(inst, "engine", None)
                engname = str(eng).split(".")[-1] if eng is not None else ""
                if (
                    si is not None
                    and si.on_wait
                    and len(si.on_wait) > 1
                    and engname in ("Activation", "PE", "DVE", "Pool", "SP")
                    and type(inst).__name__ != "InstISA"
                ):
                    waits = list(si.on_wait)
                    for k, w in enumerate(waits[:-1]):
                        nop = mybir.InstNoOp(name=f"{inst.name}-ws{k}", engine=eng)
                        nop.sync_info = mybir.SyncInfo(on_wait=[w], on_update=[])
                        insts.insert(i, nop)
                        i += 1
                    inst.sync_info = mybir.SyncInfo(
                        on_wait=[waits[-1]], on_update=list(si.on_update or [])
                    )
                    n_split += 1
                i += 1
    return nc


def _get_nc():
    if "nc" not in _CACHE:
        _CACHE["nc"] = _build_nc()
    return _CACHE["nc"]


def kernel(encoder_features, h, c, encoder_state, encoder_mask, v, W, b):
    global LAST_RESULTS
    from concourse.bass_utils import run_bass_kernel_spmd

    ef = np.ascontiguousarray(np.asarray(encoder_features, np.float32)).reshape(B, L, A)
    es = np.ascontiguousarray(np.asarray(encoder_state, np.float32)).reshape(B, L, A)
    h = np.asarray(h, np.float32)
    c = np.asarray(c, np.float32)
    mask = np.asarray(encoder_mask, np.float32)
    v = np.asarray(v, np.float32)
    W = np.asarray(W, np.float32)
    bb = np.asarray(b, np.float32)

    # dec = [h, c] @ W.T + b  (tiny: 16x1024 @ 1024x512)
    dec = np.concatenate([h, c], axis=1) @ W.T + bb  # [B, A]
    vr = np.ascontiguousarray(v.reshape(1, A))

    in_maps = []
    for k in range(NCORES):
        sl = slice(BPC * k, BPC * (k + 1))
        # maskT[p, U*b + u] = mask[b, 128u + p]
        maskT = np.ascontiguousarray(
            np.concatenate(
                [mask[BPC * k + i].reshape(U, P).T for i in range(BPC)], axis=1
            )
        )
        in_maps.append(
            {
                "feat": np.ascontiguousarray(ef[sl]),
                "state": np.ascontiguousarray(es[sl]),
                "dec": np.ascontiguousarray(dec[sl].reshape(1, BPC * A)),
                "vrow": vr,
                "maskT": maskT,
            }
        )

    nc = _get_nc()
    res = run_bass_kernel_spmd(
        nc,
        in_maps,
        core_ids=list(range(NCORES)),
        trace=TRACE or bool(int(os.environ.get("KERNEL_TRACE", "0"))),
    )
    LAST_RESULTS = res

    context = np.concatenate([r["ctx"] for r in res.results], axis=0)
    attn = np.concatenate([r["attn"] for r in res.results], axis=0)
    return context, attn
